# revision 25
# baseline (speedup 1.0000x reference)
"""Trainium2 Bass kernel for nn_Attention_3298534884255.

Computes, for inputs x:[S,B,H], hidden:[1,B,H], pad:[B,S], W,U:[H,H], v:[H,1]:
    scores[s,b] = v . tanh(hidden[0]@W [b] + (x[s,b] @ U))
    out = softmax(where(pad, -1e5, scores.T), axis=1)   -> [B, S]

Strategy: data parallelism over batch B=64 across 8 NeuronCores, PLUS
mask-aware row compaction. ~50% of pad_matrix is True and masked positions
produce exactly 0.0 in the output, so the kernel only computes scores for
unmasked (s,b) rows. The host compacts unmasked rows per batch; batches are
assigned to (core, position) by sorted count so the per-position capacity
(max across cores, required for the SPMD single-program constraint) is tight:
R = sum(caps) ~ 8.3k rows/core instead of 16384 — halving the PE matmul work,
which is the kernel bottleneck (~94% tensor-engine occupancy measured).

Per core the matmul is computed in a "proj-transposed" layout:
psum[h_out, row] = sum_k U[k,h_out] * xT[k,row], so U's natural layout is the
stationary operand and xT (host-compacted+pretransposed, fp16) streams. The
Wh bias is per-partition in this layout; a row block can straddle batch
segments, in which case the scalar-engine tanh is issued per segment (each
with its own bias). The v-dot runs on the vector engine
(scalar_tensor_tensor chain over the 8 h_out chunks) with the partition
reduction on GpSimd; the last two blocks route their reductions through the
PE (ones-matmul, then a full PE v-dot for the final block) so no serial
DVE/gpsimd chain sits exposed in the kernel tail.

The device does NOT normalize: each block's scores are exp'd (scores are
bounded by ||v||_1 ~ 25 so fp32 exp cannot overflow and no max-subtraction
is needed) into a [1, R] strip which is DMA'd back raw; the host sums each
batch's segment and divides during the scatter back to full [B,S] (exact
zeros at masked positions). This removes the on-device reduce/reciprocal/
scale/per-position-DMA tail and the whole pad-mask input: capacity-padding
rows compute garbage exp values that the host simply never reads.

Head engineering (measured): the framework preamble ends ~7us in; a HWDGE
ring delivers ~240GB/s with ~1.5us fixed cost per dma_start, and the HAM
clock-gate needs ~3.4us of sustained PE busy to release the 2.4GHz clock.
So (1) x is laid out host-side as per-block contiguous slabs so each block
is ONE line-rate transfer; (2) the first 512 rows are split into a
128/128/256 "ladder" fetched on the scalar ring while U streams on the sync
ring — the PE stream starts on a 256KB transfer (~10.5us) instead of a 1MB
one (~14us); (3) the ladder's m-loop is interleaved m-major across its
sub-blocks so U chunk consumption pacing matches a full 512-block (U m2..m7
arrive in time on the sync ring); (4) later x blocks alternate between the
two rings; (5) a short PE warmup bridges the preamble so the HAM fires just
as the stream starts.

S, B, H = 2048, 64, 1024. fp16 operands into the PE (fp32 accumulation).
NOTE: an fp8-e4m3 DoubleRow variant was tried in a previous session and
REVERTED: mixing DR-fp8 into the fp16 stream drops the whole PE stream to
~259ns/matmul (vs 216) plus a ~0.5us bubble per DR; a fully-fp8 stream
fails the accuracy budget (~2.7e-2 est. vs 2e-2 tolerance). 259ns/MM is
also the signature of the chip's P0 power-state downclock (PE 2.0GHz) —
re-measure before attributing it to a code change.
"""

import sys

import numpy as np

if "/opt/trn_rl_repo" not in sys.path:
    sys.path.insert(0, "/opt/trn_rl_repo")

import concourse.tile as tile
from concourse import bacc, bass_isa, mybir
from concourse.bass_utils import run_bass_kernel_spmd

S, B, H = 2048, 64, 1024
NCORES = 8
BLOC = B // NCORES          # batch positions per core = 8
NBLK = 512                  # rows per full PE block (one PSUM bank of fp32)
KC = H // 128               # contraction chunks = 8
MC = H // 128               # h_out chunks = 8
WARM = 85                   # PE warmup matmuls (bridge preamble -> stream)
LADDER = (512,)             # opening block, fetched scalar-ring-first
# NOTE: a 128/128/256 opening ladder was tried and REVERTED: the first
# transfer on a ring pays a ~3us cold-start penalty on top of ~4.4us/MB,
# so small opening fetches do NOT land earlier (~13us either way) — the
# ladder only added per-MM overhead and mid-ladder starvation stalls.

F16 = mybir.dt.float16
F32 = mybir.dt.float32


def _block_sizes(rtot):
    """Opening ladder then 512-row blocks; exact cover of rtot rows."""
    sizes = []
    rem = rtot
    for s in LADDER:
        if rem > s + NBLK:  # keep at least one full block after the ladder
            sizes.append(s)
            rem -= s
    while rem > NBLK:
        sizes.append(NBLK)
        rem -= NBLK
    if rem:
        sizes.append(rem)
    return sizes


def _layout(pad_matrix):
    """Sorted batch->(core,position) assignment + per-position capacities."""
    n = (~np.asarray(pad_matrix, dtype=bool)).sum(axis=1).astype(np.int64)
    order = np.argsort(-n, kind="stable")  # descending counts
    caps = [int(n[order[NCORES * b]]) for b in range(BLOC)]
    starts = [0]
    for c in caps:
        starts.append(starts[-1] + c)
    rtot = starts[-1]
    return order, caps, starts[:-1], rtot, n


def _build_program(caps, starts, rtot):
    nc = bacc.Bacc(
        "TRN2", target_bir_lowering=False, debug=False, num_devices=NCORES
    )

    xt = nc.dram_tensor("xt", [128, rtot * KC], F16, kind="ExternalInput").ap()
    ut = nc.dram_tensor("ut", [128, MC * KC * 128], F16, kind="ExternalInput").ap()
    wh = nc.dram_tensor("wh", [128, MC * BLOC], F32, kind="ExternalInput").ap()
    vv = nc.dram_tensor("vv", [128, MC], F16, kind="ExternalInput").ap()
    vvf = nc.dram_tensor("vvf", [128, MC], F32, kind="ExternalInput").ap()
    out = nc.dram_tensor("out", [1, rtot], F32, kind="ExternalOutput").ap()

    ends = [starts[b] + caps[b] for b in range(BLOC)]
    sizes = _block_sizes(rtot)
    blocks = []
    g0 = 0
    for s in sizes:
        blocks.append((g0, s))
        g0 += s
    nblocks = len(blocks)
    nladder = sum(1 for s, l in zip(sizes, LADDER) if s == l)
    # groups: the ladder blocks run as ONE m-major-interleaved group (so U
    # chunk consumption pacing matches a full 512-block); the rest singleton
    groups = ([list(range(nladder))] if nladder else []) + [
        [bi] for bi in range(nladder, nblocks)
    ]
    ngroups = len(groups)

    # per-block batch segments (a block straddles <=3 positions); the tanh
    # bias is per-batch so it is issued per segment
    blk_segs = []
    for g0b, bn in blocks:
        cur = []
        for b in range(BLOC):
            s0 = max(g0b, starts[b])
            s1 = min(g0b + bn, ends[b])
            if s1 > s0:
                cur.append((b, s0, s1))
        blk_segs.append(cur)

    with tile.TileContext(nc) as tc:
        with (
            tc.tile_pool(name="consts", bufs=1) as consts,
            tc.tile_pool(name="xblk", bufs=4) as xpool,
            tc.tile_pool(name="tanh", bufs=4) as tpool,
            tc.tile_pool(name="proj_ps", bufs=6, space="PSUM") as pspool,
            tc.tile_pool(name="score_ps", bufs=2, space="PSUM") as scpool,
            tc.tile_pool(name="softmax", bufs=1) as smpool,
        ):
            u_sb = consts.tile([128, MC * KC * 128], F16)
            ucw = KC * 128
            wh_sb = consts.tile([128, MC * BLOC], F32)
            v32_sb = consts.tile([128, MC], F32)
            v_sb = consts.tile([128, MC], F16)
            strip = consts.tile([1, rtot], F32)
            ones_sb = consts.tile([128, 1], F16)
            nc.vector.memset(ones_sb[:], 1.0)

            # x is laid out per-block on the host: block bi's slab is the
            # contiguous columns [g0*KC, (g0+bn)*KC), k-major inside
            # (xt[p, g0*KC + k*bn + n] = x[h=k*128+p, row=g0+n]), so every
            # block is ONE line-rate transfer. Sync ring: U in paced pairs
            # (u01 gates the first MM; u23/u45/u67 chase the interleaved
            # ladder's m-loop). Scalar ring: the ladder x slabs. SWDGE:
            # small consts (wh gates the first tanh, v32 the first v-dot).
            # NOTE: tiny "ring-warmup" transfers ahead of xb0 were tried and
            # REVERTED — each small leading transfer still costs ~1-1.5us of
            # serial ring time and just delays the critical xb0 delivery.
            xb_map = {}
            for bi in range(nladder):
                g0b, bn = blocks[bi]
                xl = consts.tile([128, KC * bn], F16, tag=f"xl{bi}")
                xb_map[bi] = xl
                nc.scalar.dma_start(xl[:], xt[:, g0b * KC : (g0b + bn) * KC])
            # Both rings share the ~358GB/s HBM cap during the head, so U is
            # split across them: u01 (sync) gates m0 alongside xb0 (scalar);
            # u23 rides scalar right behind xb0; u45/u67 on sync. x blocks
            # 1+ queue strictly behind all U so they can't starve it.
            nc.sync.dma_start(u_sb[:, 0 : 2 * ucw], ut[:, 0 : 2 * ucw])
            nc.scalar.dma_start(u_sb[:, 2 * ucw : 4 * ucw], ut[:, 2 * ucw : 4 * ucw])
            nc.sync.dma_start(u_sb[:, 4 * ucw : 6 * ucw], ut[:, 4 * ucw : 6 * ucw])
            nc.sync.dma_start(u_sb[:, 6 * ucw :], ut[:, 6 * ucw :])
            nc.gpsimd.dma_start(wh_sb[:], wh[:])
            nc.gpsimd.dma_start(v32_sb[:], vvf[:])
            nc.gpsimd.dma_start(v_sb[:], vv[:])

            # PE warmup: bridge the gap between the framework preamble and
            # the first x slab's arrival so the HAM activity window stays
            # busy and the 2.4GHz clock engages as the stream starts.
            warm_sb = consts.tile([128, 128], F16)
            nc.vector.memset(warm_sb[:], 0.0)
            warm_ps = pspool.tile([128, NBLK], F32, tag="pt")
            for _ in range(WARM):
                nc.tensor.matmul(
                    warm_ps[:, 0:128], warm_sb[:], warm_sb[:],
                    start=True, stop=True,
                )

            for gi, grp in enumerate(groups):
                if gi > 0:
                    bi = grp[0]
                    g0b, bn = blocks[bi]
                    xb = xpool.tile([128, KC * NBLK], F16, tag="xb")
                    xb_map[bi] = xb
                    # alternate rings: odd blocks scalar (behind the ladder
                    # slabs), even blocks sync (behind the U chunks)
                    eng = nc.scalar if bi % 2 == 1 else nc.sync
                    eng.dma_start(
                        xb[:, 0 : KC * bn], xt[:, g0b * KC : (g0b + bn) * KC]
                    )
                pe_vdot = gi == ngroups - 1
                pe_reduce = gi == ngroups - 2
                accs = {}
                ths = []
                if pe_vdot:
                    sc = scpool.tile([1, NBLK], F32, tag="sc")
                for m in range(MC):
                    for bi in grp:
                        g0b, bn = blocks[bi]
                        xb = xb_map[bi]
                        pt = pspool.tile([128, NBLK], F32, tag="pt")
                        for k in range(KC):
                            nc.tensor.matmul(
                                pt[:, 0:bn],
                                u_sb[:, (m * KC + k) * 128 : (m * KC + k + 1) * 128],
                                xb[:, k * bn : (k + 1) * bn],
                                start=(k == 0),
                                stop=(k == KC - 1),
                            )
                        if pe_vdot:
                            # all 8 th tiles stay live until the trailing
                            # sc-matmuls read them — full-depth ring
                            th = tpool.tile([128, NBLK], F16, tag="thv", bufs=MC)
                        else:
                            th = tpool.tile([128, NBLK], F16, tag="th")
                        for b, s0, s1 in blk_segs[bi]:
                            nc.scalar.activation(
                                th[:, s0 - g0b : s1 - g0b],
                                pt[:, s0 - g0b : s1 - g0b],
                                mybir.ActivationFunctionType.Tanh,
                                bias=wh_sb[:, m * BLOC + b : m * BLOC + b + 1],
                            )
                        if pe_vdot:
                            ths.append(th)
                        elif m == 0:
                            # acc = th * v[m] on the vector engine
                            acc = tpool.tile([128, NBLK], F16, tag="acc")
                            accs[bi] = acc
                            nc.vector.tensor_scalar_mul(
                                acc[:, 0:bn], th[:, 0:bn], v32_sb[:, m : m + 1]
                            )
                        else:
                            acc = accs[bi]
                            nc.vector.scalar_tensor_tensor(
                                acc[:, 0:bn],
                                th[:, 0:bn],
                                v32_sb[:, m : m + 1],
                                acc[:, 0:bn],
                                op0=mybir.AluOpType.mult,
                                op1=mybir.AluOpType.add,
                            )
                for bi in grp:
                    g0b, bn = blocks[bi]
                    if pe_vdot:
                        # v-weighted partition sum as 8 accumulating PE MMs
                        for m in range(MC):
                            nc.tensor.matmul(
                                sc[:, 0:bn],
                                v_sb[:, m : m + 1],
                                ths[m][:, 0:bn],
                                start=(m == 0),
                                stop=(m == MC - 1),
                            )
                        score_row = sc[:, 0:bn]
                    elif pe_reduce:
                        # second-to-last block: PE ones-matmul keeps the
                        # gpsimd reduce latency out of the kernel tail
                        sc = scpool.tile([1, NBLK], F32, tag="sc")
                        nc.tensor.matmul(
                            sc[:, 0:bn], ones_sb[:], accs[bi][:, 0:bn],
                            start=True, stop=True,
                        )
                        score_row = sc[:, 0:bn]
                    else:
                        # partition-sum on the (otherwise idle) GpSimd
                        red = tpool.tile([128, NBLK], F32, tag="red")
                        nc.gpsimd.partition_all_reduce(
                            red[:, 0:bn], accs[bi][:, 0:bn], 128,
                            bass_isa.ReduceOp.add,
                        )
                        score_row = red[0:1, 0:bn]
                    # exp straight into the strip; normalization happens on
                    # the host, so no accumulators and no pad masking:
                    # capacity-pad rows produce garbage exp values the host
                    # never reads
                    nc.scalar.activation(
                        strip[:, g0b : g0b + bn],
                        score_row,
                        mybir.ActivationFunctionType.Exp,
                    )
                    if pe_reduce:
                        # everything before the final block is exp'd: stream
                        # the bulk of the strip out now (issued from the
                        # scalar queue — the exp just ran on this engine, so
                        # no cross-engine hop), leaving only the last
                        # block's sliver for the kernel tail
                        nc.scalar.dma_start(
                            out[0:1, 0 : g0b + bn], strip[:, 0 : g0b + bn]
                        )
                    if pe_vdot:
                        nc.scalar.dma_start(
                            out[0:1, g0b : g0b + bn], strip[:, g0b : g0b + bn]
                        )

    nc.compile()
    return nc


_NC = None
_NC_KEY = None
_LAYOUT = None


def _get_program():
    assert _NC is not None, "call _prepare_in_maps first"
    return _NC


def _prepare_in_maps(inputs, hidden, pad_matrix, W, U, v):
    global _NC, _NC_KEY, _LAYOUT
    inputs = np.asarray(inputs, dtype=np.float32)
    hidden = np.asarray(hidden, dtype=np.float32)
    pad_matrix = np.asarray(pad_matrix, dtype=bool)
    W = np.asarray(W, dtype=np.float32)
    U = np.asarray(U, dtype=np.float32)
    v = np.asarray(v, dtype=np.float32)

    order, caps, starts, rtot, n = _layout(pad_matrix)
    _LAYOUT = (order, caps, starts, rtot, n, pad_matrix)
    key = (rtot, tuple(caps))
    if _NC is None or _NC_KEY != key:
        _NC = _build_program(caps, starts, rtot)
        _NC_KEY = key

    # xT_all[h, b, s] = inputs[s, b, h], fp16
    xt_all = np.ascontiguousarray(inputs.transpose(2, 1, 0)).astype(np.float16)
    # U tiled m-major: ut[p, ((m*KC + k)*128 + j)] = U[k*128+p, m*128+j]
    ut = np.ascontiguousarray(
        U.reshape(KC, 128, MC, 128).transpose(1, 2, 0, 3)
    ).reshape(128, MC * KC * 128).astype(np.float16)
    # bias Wh = hidden[0] @ W, fp32 on host (0.05% of total FLOPs)
    Wh = hidden[0] @ W  # [B, H]
    # v tiled: vv[p, m] = v[m*128+p]
    vvf = np.ascontiguousarray(v[:, 0].reshape(MC, 128).T).astype(np.float32)
    vv = vvf.astype(np.float16)

    sizes = _block_sizes(rtot)
    blocks = []
    g0 = 0
    for s in sizes:
        blocks.append((g0, s))
        g0 += s

    in_maps = []
    for c in range(NCORES):
        xt_c = np.zeros((H, rtot), dtype=np.float16)
        wh_c = np.empty((BLOC, H), dtype=np.float32)
        for b in range(BLOC):
            batch = int(order[NCORES * b + c])
            idx = np.flatnonzero(~pad_matrix[batch])
            nb = len(idx)
            xt_c[:, starts[b] : starts[b] + nb] = xt_all[:, batch, idx]
            wh_c[b] = Wh[batch]
        # per-block contiguous slabs, k-major inside each block:
        # xt_blk[p, g0*KC + k*bn + n] = xt_c[k*128+p, g0+n]
        x_k = xt_c.reshape(KC, 128, rtot)
        slabs = [
            np.ascontiguousarray(x_k[:, :, g0b : g0b + bn].transpose(1, 0, 2))
            .reshape(128, KC * bn)
            for (g0b, bn) in blocks
        ]
        xt_blk = np.concatenate(slabs, axis=1)
        # wh[p, m*BLOC + b] = Wh[batch(c,b), m*128+p]
        wh_t = np.ascontiguousarray(
            wh_c.reshape(BLOC, MC, 128).transpose(2, 1, 0)
        ).reshape(128, MC * BLOC)
        in_maps.append(
            {"xt": xt_blk, "ut": ut, "wh": wh_t, "vv": vv, "vvf": vvf}
        )
    return in_maps


def _postprocess(results):
    order, caps, starts, rtot, n, pad_matrix = _LAYOUT
    out_full = np.zeros((B, S), dtype=np.float32)
    for c in range(NCORES):
        o = np.asarray(results[c]["out"], dtype=np.float32).reshape(rtot)
        for b in range(BLOC):
            batch = int(order[NCORES * b + c])
            idx = np.flatnonzero(~pad_matrix[batch])
            if len(idx) == 0:
                # all-masked row: reference softmax degenerates to uniform
                out_full[batch, :] = 1.0 / S
                continue
            vals = o[starts[b] : starts[b] + len(idx)].astype(np.float64)
            out_full[batch, idx] = (vals / vals.sum()).astype(np.float32)
    return out_full


def kernel(inputs, hidden, pad_matrix, W, U, v):
    if not (~np.asarray(pad_matrix, dtype=bool)).any():
        # fully masked: softmax of a constant row -> uniform
        return np.full((B, S), 1.0 / S, dtype=np.float32)
    in_maps = _prepare_in_maps(inputs, hidden, pad_matrix, W, U, v)
    nc = _get_program()
    res = run_bass_kernel_spmd(nc, in_maps, core_ids=list(range(NCORES)))
    return _postprocess(res.results)


# revision 26
# speedup vs baseline: 1.1867x; 1.1867x over previous
"""Trainium2 Bass kernel for nn_Attention_3298534884255.

Computes, for inputs x:[S,B,H], hidden:[1,B,H], pad:[B,S], W,U:[H,H], v:[H,1]:
    scores[s,b] = v . tanh(hidden[0]@W [b] + (x[s,b] @ U))
    out = softmax(where(pad, -1e5, scores.T), axis=1)   -> [B, S]

Strategy: data parallelism over batch B=64 across 8 NeuronCores, PLUS
mask-aware row compaction. ~50% of pad_matrix is True and masked positions
produce exactly 0.0 in the output, so the kernel only computes scores for
unmasked (s,b) rows. The host compacts unmasked rows per batch; batches are
assigned to (core, position) by sorted count so the per-position capacity
(max across cores, required for the SPMD single-program constraint) is tight:
R = sum(caps) ~ 8.3k rows/core instead of 16384 — halving the PE matmul work,
which is the kernel bottleneck (~94% tensor-engine occupancy measured).

Per core the matmul is computed in a "proj-transposed" layout:
psum[h_out, row] = sum_k U[k,h_out] * xT[k,row], so U's natural layout is the
stationary operand and xT (host-compacted+pretransposed, fp16) streams. The
Wh bias is per-partition in this layout; a row block can straddle batch
segments, in which case the scalar-engine tanh is issued per segment (each
with its own bias). The v-dot runs on the vector engine
(scalar_tensor_tensor chain over the 8 h_out chunks) with the partition
reduction on GpSimd; the last two blocks route their reductions through the
PE (ones-matmul, then a full PE v-dot for the final block) so no serial
DVE/gpsimd chain sits exposed in the kernel tail.

The device does NOT normalize: each block's scores are exp'd (scores are
bounded by ||v||_1 ~ 25 so fp32 exp cannot overflow and no max-subtraction
is needed) into a [1, R] strip which is DMA'd back raw; the host sums each
batch's segment and divides during the scatter back to full [B,S] (exact
zeros at masked positions). This removes the on-device reduce/reciprocal/
scale/per-position-DMA tail and the whole pad-mask input: capacity-padding
rows compute garbage exp values that the host simply never reads.

Head engineering (measured): the framework preamble ends ~7us in; a HWDGE
ring delivers ~240GB/s with ~1.5us fixed cost per dma_start, and the HAM
clock-gate needs ~3.4us of sustained PE busy to release the 2.4GHz clock.
So (1) x is laid out host-side as per-block contiguous slabs so each block
is ONE line-rate transfer; (2) the first 512 rows are split into a
128/128/256 "ladder" fetched on the scalar ring while U streams on the sync
ring — the PE stream starts on a 256KB transfer (~10.5us) instead of a 1MB
one (~14us); (3) the ladder's m-loop is interleaved m-major across its
sub-blocks so U chunk consumption pacing matches a full 512-block (U m2..m7
arrive in time on the sync ring); (4) later x blocks alternate between the
two rings; (5) a short PE warmup bridges the preamble so the HAM fires just
as the stream starts.

S, B, H = 2048, 64, 1024. fp16 operands into the PE (fp32 accumulation).
NOTE: an fp8-e4m3 DoubleRow variant was tried in a previous session and
REVERTED: mixing DR-fp8 into the fp16 stream drops the whole PE stream to
~259ns/matmul (vs 216) plus a ~0.5us bubble per DR; a fully-fp8 stream
fails the accuracy budget (~2.7e-2 est. vs 2e-2 tolerance). 259ns/MM is
also the signature of the chip's P0 power-state downclock (PE 2.0GHz) —
re-measure before attributing it to a code change.
"""

import sys

import numpy as np

if "/opt/trn_rl_repo" not in sys.path:
    sys.path.insert(0, "/opt/trn_rl_repo")

import concourse.tile as tile
from concourse import bacc, bass_isa, mybir
from concourse.bass_utils import run_bass_kernel_spmd

S, B, H = 2048, 64, 1024
NCORES = 8
BLOC = B // NCORES          # batch positions per core = 8
NBLK = 512                  # rows per full PE block (one PSUM bank of fp32)
KC = H // 128               # contraction chunks = 8
MC = H // 128               # h_out chunks = 8
WARM = 100                  # PE warmup matmuls: sized so warmup ends ~15.3us,
                            # the MEAN observed block-0 x arrival (idle-vs-
                            # delay cost is symmetric, so aim at the mean)
LADDER = (512,)             # opening block, fetched scalar-ring-first
# NOTE: a 128/128/256 opening ladder was tried and REVERTED: the first
# transfer on a ring pays a ~3us cold-start penalty on top of ~4.4us/MB,
# so small opening fetches do NOT land earlier (~13us either way) — the
# ladder only added per-MM overhead and mid-ladder starvation stalls.

F16 = mybir.dt.float16
F32 = mybir.dt.float32


def _block_sizes(rtot):
    """Opening ladder then 512-row blocks; exact cover of rtot rows."""
    sizes = []
    rem = rtot
    for s in LADDER:
        if rem > s + NBLK:  # keep at least one full block after the ladder
            sizes.append(s)
            rem -= s
    while rem > NBLK:
        sizes.append(NBLK)
        rem -= NBLK
    if rem:
        sizes.append(rem)
    return sizes


def _layout(pad_matrix):
    """Sorted batch->(core,position) assignment + per-position capacities."""
    n = (~np.asarray(pad_matrix, dtype=bool)).sum(axis=1).astype(np.int64)
    order = np.argsort(-n, kind="stable")  # descending counts
    caps = [int(n[order[NCORES * b]]) for b in range(BLOC)]
    starts = [0]
    for c in caps:
        starts.append(starts[-1] + c)
    rtot = starts[-1]
    return order, caps, starts[:-1], rtot, n


def _build_program(caps, starts, rtot):
    nc = bacc.Bacc(
        "TRN2", target_bir_lowering=False, debug=False, num_devices=NCORES
    )

    xt = nc.dram_tensor("xt", [128, rtot * KC], F16, kind="ExternalInput").ap()
    ut = nc.dram_tensor("ut", [128, MC * KC * 128], F16, kind="ExternalInput").ap()
    wh = nc.dram_tensor("wh", [128, MC * BLOC], F32, kind="ExternalInput").ap()
    vv = nc.dram_tensor("vv", [128, MC], F16, kind="ExternalInput").ap()
    vvf = nc.dram_tensor("vvf", [128, MC], F32, kind="ExternalInput").ap()
    out = nc.dram_tensor("out", [1, rtot], F32, kind="ExternalOutput").ap()

    ends = [starts[b] + caps[b] for b in range(BLOC)]
    sizes = _block_sizes(rtot)
    blocks = []
    g0 = 0
    for s in sizes:
        blocks.append((g0, s))
        g0 += s
    nblocks = len(blocks)
    nladder = sum(1 for s, l in zip(sizes, LADDER) if s == l)
    # groups: the ladder blocks run as ONE m-major-interleaved group (so U
    # chunk consumption pacing matches a full 512-block); the rest singleton
    groups = ([list(range(nladder))] if nladder else []) + [
        [bi] for bi in range(nladder, nblocks)
    ]
    ngroups = len(groups)

    # per-block batch segments (a block straddles <=3 positions); the tanh
    # bias is per-batch so it is issued per segment
    blk_segs = []
    for g0b, bn in blocks:
        cur = []
        for b in range(BLOC):
            s0 = max(g0b, starts[b])
            s1 = min(g0b + bn, ends[b])
            if s1 > s0:
                cur.append((b, s0, s1))
        blk_segs.append(cur)

    with tile.TileContext(nc) as tc:
        with (
            tc.tile_pool(name="consts", bufs=1) as consts,
            tc.tile_pool(name="xblk", bufs=4) as xpool,
            tc.tile_pool(name="tanh", bufs=4) as tpool,
            tc.tile_pool(name="proj_ps", bufs=6, space="PSUM") as pspool,
            tc.tile_pool(name="score_ps", bufs=2, space="PSUM") as scpool,
            tc.tile_pool(name="softmax", bufs=1) as smpool,
        ):
            u_sb = consts.tile([128, MC * KC * 128], F16)
            ucw = KC * 128
            wh_sb = consts.tile([128, MC * BLOC], F32)
            v32_sb = consts.tile([128, MC], F32)
            v_sb = consts.tile([128, MC], F16)
            strip = consts.tile([1, rtot], F32)
            ones_sb = consts.tile([128, 1], F16)
            nc.vector.memset(ones_sb[:], 1.0)

            # x is laid out per-block on the host: block bi's slab is the
            # contiguous columns [g0*KC, (g0+bn)*KC), k-major inside
            # (xt[p, g0*KC + k*bn + n] = x[h=k*128+p, row=g0+n]), so every
            # block is ONE line-rate transfer. Sync ring: U in paced pairs
            # (u01 gates the first MM; u23/u45/u67 chase the interleaved
            # ladder's m-loop). Scalar ring: the ladder x slabs. SWDGE:
            # small consts (wh gates the first tanh, v32 the first v-dot).
            # NOTE: tiny "ring-warmup" transfers ahead of xb0 were tried and
            # REVERTED — each small leading transfer still costs ~1-1.5us of
            # serial ring time and just delays the critical xb0 delivery.
            xb_map = {}
            for bi in range(nladder):
                g0b, bn = blocks[bi]
                xl = consts.tile([128, KC * bn], F16, tag=f"xl{bi}")
                xb_map[bi] = xl
                nc.scalar.dma_start(xl[:], xt[:, g0b * KC : (g0b + bn) * KC])
            # Both rings share the ~358GB/s HBM cap during the head, so U is
            # split across them: u01 (sync) gates m0 alongside xb0 (scalar);
            # u23 rides scalar right behind xb0; u45/u67 on sync. x blocks
            # 1+ queue strictly behind all U so they can't starve it.
            nc.sync.dma_start(u_sb[:, 0 : 2 * ucw], ut[:, 0 : 2 * ucw])
            nc.scalar.dma_start(u_sb[:, 2 * ucw : 4 * ucw], ut[:, 2 * ucw : 4 * ucw])
            nc.sync.dma_start(u_sb[:, 4 * ucw : 6 * ucw], ut[:, 4 * ucw : 6 * ucw])
            nc.sync.dma_start(u_sb[:, 6 * ucw :], ut[:, 6 * ucw :])
            nc.gpsimd.dma_start(wh_sb[:], wh[:])
            nc.gpsimd.dma_start(v32_sb[:], vvf[:])
            nc.gpsimd.dma_start(v_sb[:], vv[:])

            # PE warmup: bridge the gap between the framework preamble and
            # the first x slab's arrival so the HAM activity window stays
            # busy and the 2.4GHz clock engages as the stream starts.
            warm_sb = consts.tile([128, 128], F16)
            nc.vector.memset(warm_sb[:], 0.0)
            warm_ps = pspool.tile([128, NBLK], F32, tag="pt")
            for _ in range(WARM):
                nc.tensor.matmul(
                    warm_ps[:, 0:128], warm_sb[:], warm_sb[:],
                    start=True, stop=True,
                )

            for gi, grp in enumerate(groups):
                if gi > 0:
                    bi = grp[0]
                    g0b, bn = blocks[bi]
                    xb = xpool.tile([128, KC * NBLK], F16, tag="xb")
                    xb_map[bi] = xb
                    # alternate rings: odd blocks scalar (behind the ladder
                    # slabs), even blocks sync (behind the U chunks)
                    eng = nc.scalar if bi % 2 == 1 else nc.sync
                    eng.dma_start(
                        xb[:, 0 : KC * bn], xt[:, g0b * KC : (g0b + bn) * KC]
                    )
                pe_vdot = gi == ngroups - 1
                pe_reduce = gi == ngroups - 2
                accs = {}
                ths = []
                if pe_vdot:
                    sc = scpool.tile([1, NBLK], F32, tag="sc")
                for m in range(MC):
                    for bi in grp:
                        g0b, bn = blocks[bi]
                        xb = xb_map[bi]
                        pt = pspool.tile([128, NBLK], F32, tag="pt")
                        for k in range(KC):
                            nc.tensor.matmul(
                                pt[:, 0:bn],
                                u_sb[:, (m * KC + k) * 128 : (m * KC + k + 1) * 128],
                                xb[:, k * bn : (k + 1) * bn],
                                start=(k == 0),
                                stop=(k == KC - 1),
                            )
                        if pe_vdot:
                            # all 8 th tiles stay live until the trailing
                            # sc-matmuls read them — full-depth ring
                            th = tpool.tile([128, NBLK], F16, tag="thv", bufs=MC)
                        else:
                            th = tpool.tile([128, NBLK], F16, tag="th")
                        for b, s0, s1 in blk_segs[bi]:
                            nc.scalar.activation(
                                th[:, s0 - g0b : s1 - g0b],
                                pt[:, s0 - g0b : s1 - g0b],
                                mybir.ActivationFunctionType.Tanh,
                                bias=wh_sb[:, m * BLOC + b : m * BLOC + b + 1],
                            )
                        if pe_vdot:
                            ths.append(th)
                        elif m == 0:
                            # acc = th * v[m] on the vector engine
                            acc = tpool.tile([128, NBLK], F16, tag="acc")
                            accs[bi] = acc
                            nc.vector.tensor_scalar_mul(
                                acc[:, 0:bn], th[:, 0:bn], v32_sb[:, m : m + 1]
                            )
                        else:
                            acc = accs[bi]
                            nc.vector.scalar_tensor_tensor(
                                acc[:, 0:bn],
                                th[:, 0:bn],
                                v32_sb[:, m : m + 1],
                                acc[:, 0:bn],
                                op0=mybir.AluOpType.mult,
                                op1=mybir.AluOpType.add,
                            )
                for bi in grp:
                    g0b, bn = blocks[bi]
                    if pe_vdot:
                        # v-weighted partition sum as 8 accumulating PE MMs
                        for m in range(MC):
                            nc.tensor.matmul(
                                sc[:, 0:bn],
                                v_sb[:, m : m + 1],
                                ths[m][:, 0:bn],
                                start=(m == 0),
                                stop=(m == MC - 1),
                            )
                        score_row = sc[:, 0:bn]
                    elif pe_reduce:
                        # second-to-last block: PE ones-matmul keeps the
                        # gpsimd reduce latency out of the kernel tail
                        sc = scpool.tile([1, NBLK], F32, tag="sc")
                        nc.tensor.matmul(
                            sc[:, 0:bn], ones_sb[:], accs[bi][:, 0:bn],
                            start=True, stop=True,
                        )
                        score_row = sc[:, 0:bn]
                    else:
                        # partition-sum on the (otherwise idle) GpSimd
                        red = tpool.tile([128, NBLK], F32, tag="red")
                        nc.gpsimd.partition_all_reduce(
                            red[:, 0:bn], accs[bi][:, 0:bn], 128,
                            bass_isa.ReduceOp.add,
                        )
                        score_row = red[0:1, 0:bn]
                    # exp straight into the strip; normalization happens on
                    # the host, so no accumulators and no pad masking:
                    # capacity-pad rows produce garbage exp values the host
                    # never reads
                    nc.scalar.activation(
                        strip[:, g0b : g0b + bn],
                        score_row,
                        mybir.ActivationFunctionType.Exp,
                    )
                    if pe_reduce:
                        # everything before the final block is exp'd: stream
                        # the bulk of the strip out now (issued from the
                        # scalar queue — the exp just ran on this engine, so
                        # no cross-engine hop), leaving only the last
                        # block's sliver for the kernel tail
                        nc.scalar.dma_start(
                            out[0:1, 0 : g0b + bn], strip[:, 0 : g0b + bn]
                        )
                    if pe_vdot:
                        nc.scalar.dma_start(
                            out[0:1, g0b : g0b + bn], strip[:, g0b : g0b + bn]
                        )

    nc.compile()
    return nc


_NC = None
_NC_KEY = None
_LAYOUT = None


def _get_program():
    assert _NC is not None, "call _prepare_in_maps first"
    return _NC


def _prepare_in_maps(inputs, hidden, pad_matrix, W, U, v):
    global _NC, _NC_KEY, _LAYOUT
    inputs = np.asarray(inputs, dtype=np.float32)
    hidden = np.asarray(hidden, dtype=np.float32)
    pad_matrix = np.asarray(pad_matrix, dtype=bool)
    W = np.asarray(W, dtype=np.float32)
    U = np.asarray(U, dtype=np.float32)
    v = np.asarray(v, dtype=np.float32)

    order, caps, starts, rtot, n = _layout(pad_matrix)
    _LAYOUT = (order, caps, starts, rtot, n, pad_matrix)
    key = (rtot, tuple(caps))
    if _NC is None or _NC_KEY != key:
        _NC = _build_program(caps, starts, rtot)
        _NC_KEY = key

    # xT_all[h, b, s] = inputs[s, b, h], fp16
    xt_all = np.ascontiguousarray(inputs.transpose(2, 1, 0)).astype(np.float16)
    # U tiled m-major: ut[p, ((m*KC + k)*128 + j)] = U[k*128+p, m*128+j]
    ut = np.ascontiguousarray(
        U.reshape(KC, 128, MC, 128).transpose(1, 2, 0, 3)
    ).reshape(128, MC * KC * 128).astype(np.float16)
    # bias Wh = hidden[0] @ W, fp32 on host (0.05% of total FLOPs)
    Wh = hidden[0] @ W  # [B, H]
    # v tiled: vv[p, m] = v[m*128+p]
    vvf = np.ascontiguousarray(v[:, 0].reshape(MC, 128).T).astype(np.float32)
    vv = vvf.astype(np.float16)

    sizes = _block_sizes(rtot)
    blocks = []
    g0 = 0
    for s in sizes:
        blocks.append((g0, s))
        g0 += s

    in_maps = []
    for c in range(NCORES):
        xt_c = np.zeros((H, rtot), dtype=np.float16)
        wh_c = np.empty((BLOC, H), dtype=np.float32)
        for b in range(BLOC):
            batch = int(order[NCORES * b + c])
            idx = np.flatnonzero(~pad_matrix[batch])
            nb = len(idx)
            xt_c[:, starts[b] : starts[b] + nb] = xt_all[:, batch, idx]
            wh_c[b] = Wh[batch]
        # per-block contiguous slabs, k-major inside each block:
        # xt_blk[p, g0*KC + k*bn + n] = xt_c[k*128+p, g0+n]
        x_k = xt_c.reshape(KC, 128, rtot)
        slabs = [
            np.ascontiguousarray(x_k[:, :, g0b : g0b + bn].transpose(1, 0, 2))
            .reshape(128, KC * bn)
            for (g0b, bn) in blocks
        ]
        xt_blk = np.concatenate(slabs, axis=1)
        # wh[p, m*BLOC + b] = Wh[batch(c,b), m*128+p]
        wh_t = np.ascontiguousarray(
            wh_c.reshape(BLOC, MC, 128).transpose(2, 1, 0)
        ).reshape(128, MC * BLOC)
        in_maps.append(
            {"xt": xt_blk, "ut": ut, "wh": wh_t, "vv": vv, "vvf": vvf}
        )
    return in_maps


def _postprocess(results):
    order, caps, starts, rtot, n, pad_matrix = _LAYOUT
    out_full = np.zeros((B, S), dtype=np.float32)
    for c in range(NCORES):
        o = np.asarray(results[c]["out"], dtype=np.float32).reshape(rtot)
        for b in range(BLOC):
            batch = int(order[NCORES * b + c])
            idx = np.flatnonzero(~pad_matrix[batch])
            if len(idx) == 0:
                # all-masked row: reference softmax degenerates to uniform
                out_full[batch, :] = 1.0 / S
                continue
            vals = o[starts[b] : starts[b] + len(idx)].astype(np.float64)
            out_full[batch, idx] = (vals / vals.sum()).astype(np.float32)
    return out_full


def kernel(inputs, hidden, pad_matrix, W, U, v):
    if not (~np.asarray(pad_matrix, dtype=bool)).any():
        # fully masked: softmax of a constant row -> uniform
        return np.full((B, S), 1.0 / S, dtype=np.float32)
    in_maps = _prepare_in_maps(inputs, hidden, pad_matrix, W, U, v)
    nc = _get_program()
    res = run_bass_kernel_spmd(nc, in_maps, core_ids=list(range(NCORES)))
    return _postprocess(res.results)
